# revision 38
# baseline (speedup 1.0000x reference)
"""Trainium2 Bass kernel for causal multi-head attention.

Problem: X[2, 2048, 1024] fp32, 16 heads x head_dim 64, causal softmax,
out = softmax(QK^T/sqrt(64)) V @ Wo + bo.

Sharding: batch x head tensor parallel. Core c gets batch c//4 and the 4
heads 4*(c%4)..4*(c%4)+4 (two "pairs" of 2 heads): Wq/Wk/Wv column slices
[1024, 256], Wo row slice [256, 1024]. X^T (host pre-transposed, bf16)
replicated within each batch group. Each core returns a bf16 partial
[2048, 1024]; host sums 4 partials per batch + bias in fp32.

Per-core dataflow (all bf16 matmuls; cost model charges ap_size rows only):
  Q^T,K^T [128, 2048]  <- W-stationary MMs over X^T   (PSUM->SBUF bf16)
  V[k, 256]            <- X^T-stationary MMs -> natural [k, 4*64] layout
  S^T[k, q]            <- per (pair, 1024-q-group, kt): K^T chunk x Q^T
  e = exp(scale*S^T)   <- one wide ACT instr per (head, kt), diag masked
                          on Pool (affine_select)
  ctx[q, dh]           <- flipped MMs: lhsT = e block [128k, 128q]
                          (stationary), rhs = V slice; PSUM stripes
                          accumulate across kt
  sums                 <- ap-1 MMs (rhs = ones col) into rotating PSUM
                          quarter-slots, DVE-accumulated into SBUF
  normalize            <- DVE reciprocal of sums + per-partition
                          tensor_scalar mult -> ctx_n [128q, 128dh] bf16
  ctxT                 <- DMA-XBAR transpose ctx_n -> per-qt ctxT [dh, q]
  out[q, 1024]         <- ctxT (lhsT) x Wo chunks, 2-chunk PSUM accum,
                          bf16 partial DMA'd out (SP queue)

PSUM banks (8): scores h0 (2), scores h1 (2), ctx h0 (1), ctx h1 (1),
op0 (1), op1 (1). op0/op1 carry 4 rotating 256-col quarter-slots shared
by QKV projection chunks, out-projection quarters, and sums partials.

PSUM accumulation semantics (hardware-verified): start=True clears the
accumulate-bits of the WHOLE bank; each address's first write after that
overwrites, later writes accumulate. So every accumulation group in a
bank must be contiguous on the PE queue and start with start=True, OR
(ctx stripes) the bank gets exactly one start for the whole sweep.

Emission is software-pipelined: attn@V lags scores by one kt step, and
QKV / out-projection chunks are injected between attention steps so the
PE fills the gaps while ACT (the exp engine) streams.
"""

from contextlib import ExitStack

import numpy as np
import ml_dtypes

import concourse.bass as bass
import concourse.mybir as mybir
import concourse.tile as tile
from concourse import bacc
from concourse.bass_utils import run_bass_kernel_spmd
from concourse.masks import make_identity

N, D = 2048, 1024
NCORES = 8
HPC = 4                      # heads per core
DPC = HPC * 64               # projection cols per core = 256
DH = 64
P = 128
DC = D // P                  # 8 contraction chunks
NT = N // P                  # 16 n/k tiles
SCALE = 1.0 / float(np.sqrt(DH))

BF16 = mybir.dt.bfloat16
F32 = mybir.dt.float32


def build_nc() -> bass.Bass:
    nc = bacc.Bacc("TRN2", target_bir_lowering=False, debug=False)

    x = nc.dram_tensor("x", [D, N], BF16, kind="ExternalInput")
    wq = nc.dram_tensor("wq", [D, DPC], BF16, kind="ExternalInput")
    wk = nc.dram_tensor("wk", [D, DPC], BF16, kind="ExternalInput")
    wv = nc.dram_tensor("wv", [D, DPC], BF16, kind="ExternalInput")
    wo = nc.dram_tensor("wo", [DPC, D], BF16, kind="ExternalInput")
    out = nc.dram_tensor("out", [N, D], BF16, kind="ExternalOutput")

    with tile.TileContext(nc) as tc, ExitStack() as ctx:
        consts = ctx.enter_context(tc.tile_pool(name="consts", bufs=1))
        xt_pool = ctx.enter_context(tc.tile_pool(name="xt", bufs=1))
        qk_pool = ctx.enter_context(tc.tile_pool(name="qk", bufs=1))
        v_pool = ctx.enter_context(tc.tile_pool(name="v", bufs=1))
        e_pool = ctx.enter_context(tc.tile_pool(name="e", bufs=8))
        ctxT_pool = ctx.enter_context(tc.tile_pool(name="ctxT", bufs=1))
        ctxn_pool = ctx.enter_context(tc.tile_pool(name="ctxn", bufs=4))
        sums_pool = ctx.enter_context(tc.tile_pool(name="sums", bufs=2))
        rcp_pool = ctx.enter_context(tc.tile_pool(name="rcp", bufs=4))
        o_pool = ctx.enter_context(tc.tile_pool(name="o", bufs=6))
        ps = ctx.enter_context(tc.tile_pool(name="ps", bufs=1, space="PSUM"))

        # --- constants / weights + X^T, split across the two HWDGE queues
        # (SP + ACT) so the startup-critical pieces land early ---
        wq_sb = consts.tile([P, DC, DPC], BF16, tag="wq")
        wk_sb = consts.tile([P, DC, DPC], BF16, tag="wk")
        wv_sb = consts.tile([P, DC, DPC], BF16, tag="wv")
        wo_sb = consts.tile([P, 2, D], BF16, tag="wo")
        ones_sb = consts.tile([P, 1], BF16, tag="ones")
        ident_sb = consts.tile([P, P], BF16, tag="ident")
        nc.gpsimd.memset(ones_sb[:], 1.0)
        make_identity(nc, ident_sb[:])

        xt = xt_pool.tile([P, DC, N], BF16, tag="xt")
        xrr = x[:].rearrange("(dc p) n -> p dc n", p=P)
        # HWDGE setup (~625ns) does not overlap the transfer on a queue,
        # so use few, large DMAs. ACT queue: weights (wq first). SP queue:
        # X^T in increasing-n chunks; first mm needs only wq + x[0:256].
        nc.scalar.dma_start(wq_sb[:], wq[:].rearrange("(dc p) m -> p dc m", p=P))
        nc.scalar.dma_start(wv_sb[:], wv[:].rearrange("(dc p) m -> p dc m", p=P))
        nc.scalar.dma_start(wk_sb[:], wk[:].rearrange("(dc p) m -> p dc m", p=P))
        nc.scalar.dma_start(wo_sb[:], wo[:].rearrange("(c p) d -> p c d", p=P))
        for c0, c1 in ((0, 256), (256, 512), (512, 1024), (1024, 1536), (1536, 2048)):
            nc.sync.dma_start(xt[:, :, c0:c1], xrr[:, :, c0:c1])

        # SBUF destinations
        qT = [qk_pool.tile([P, N], BF16, tag=f"qT{p}", name=f"qT{p}") for p in range(2)]
        kT = [qk_pool.tile([P, N], BF16, tag=f"kT{p}", name=f"kT{p}") for p in range(2)]
        v = v_pool.tile([P, NT, DPC], BF16, tag="v")
        # per-qt ctxT tiles: the DMA-XBAR transpose writes whole tiles so
        # write coverage is registered exactly.
        ctxT = [[ctxT_pool.tile([P, P], BF16, tag=f"ctxT{p}_{qt}",
                                name=f"ctxT{p}_{qt}") for qt in range(NT)]
                for p in range(2)]

        # ---------- rotating PSUM quarter-slots ----------
        op_banks = [ps.tile([P, 2, 256], F32, tag=f"op{i}", name=f"op{i}")
                    for i in range(2)]
        rr = [0]

        def op_slot():
            i = rr[0]
            rr[0] = (i + 1) % 4
            return op_banks[i // 2][:, i % 2]

        def emit_qk_half(p2, proj, qg, half):
            """Project q or k for pair p2, n-columns [qg*512+half*256 ...+256]."""
            w_sb = wq_sb if proj == "q" else wk_sb
            dst = qT[p2] if proj == "q" else kT[p2]
            sl = slice(qg * 512 + half * 256, qg * 512 + half * 256 + 256)
            acc = op_slot()
            for dc in range(DC):
                nc.tensor.matmul(
                    acc, w_sb[:, dc, 128 * p2:128 * p2 + 128],
                    xt[:, dc, sl], start=(dc == 0), stop=(dc == DC - 1),
                    skip_group_check=True)
            nc.vector.tensor_copy(dst[:, sl], acc)

        def emit_qk(p2, proj, qg):
            for half in range(2):
                emit_qk_half(p2, proj, qg, half)

        def emit_v(nt):
            """V natural layout for n-tile nt: [128 k, 256 dh] (all 4 heads)."""
            acc = op_slot()
            for dc in range(DC):
                nc.tensor.matmul(
                    acc, xt[:, dc, nt * P:(nt + 1) * P],
                    wv_sb[:, dc, :], start=(dc == 0), stop=(dc == DC - 1),
                    skip_group_check=True)
            nc.vector.tensor_copy(v[:, nt, :], acc)

        def emit_outproj(qt, tail=False):
            """Out-projection for q-tile qt: 4 quarter-columns of 256.
            tail=True: past the last exp — alternate copies onto the
            otherwise-idle ACT engine."""
            for half in range(2):
                o_sb = o_pool.tile([P, 512], BF16, tag="o", name="o_sb")
                for sub in range(2):
                    quarter = 2 * half + sub
                    acc = op_slot()
                    for p2 in range(2):
                        nc.tensor.matmul(
                            acc, ctxT[p2][qt][:],
                            wo_sb[:, p2, quarter * 256:(quarter + 1) * 256],
                            start=(p2 == 0), stop=(p2 == 1),
                            skip_group_check=True)
                    dst = o_sb[:, sub * 256:(sub + 1) * 256]
                    if tail and sub == 1:
                        nc.scalar.copy(dst, acc)
                    else:
                        nc.vector.tensor_copy(dst, acc)
                nc.sync.dma_start(
                    out[qt * P:(qt + 1) * P, half * 512:(half + 1) * 512],
                    o_sb[:])

        # ---------- attention sweep ----------
        def emit_sweep(p2, G, inject, pinned, post=None, tail=False):
            """Attention for pair p2 over q in [G*1024, (G+1)*1024).
            inject: closures spread over steps; pinned: step -> closures
            run before attnv; post: step -> closures run after that
            step's finisher (consumers of this sweep's ctxT go here).
            tail=True: last sweep of the kernel — use the low-latency
            PE-transpose finisher for the final q-tiles."""
            post = post or {}
            nkt = 8 * G + 8
            cx = [ps.tile([P, 8, DH], F32, tag=f"cx{h}", name=f"cx{h}")
                  for h in range(2)]
            # SBUF softmax-sum accumulators, qsub-major h-minor [128, 16]
            sums = sums_pool.tile([P, 16], F32, tag="sums", name="sums")
            es = {}
            n_steps = nkt + 1
            share = [[] for _ in range(n_steps)]
            for i, f in enumerate(inject):
                share[min(1 + i * n_steps // max(1, len(inject)), n_steps - 1)].append(f)

            def scores_exp(kt):
                cs = max(0, kt * P - G * 1024)
                for h in range(2):
                    sc = ps.tile([P, 1024], F32, tag=f"sc{h}", name=f"sc{h}")
                    for c0, c1 in ((cs, 512), (max(cs, 512), 1024)):
                        if c0 >= c1:
                            continue
                        nc.tensor.matmul(
                            sc[:, c0:c1],
                            kT[p2][64 * h:64 * h + 64, kt * P:(kt + 1) * P],
                            qT[p2][64 * h:64 * h + 64, G * 1024 + c0:G * 1024 + c1],
                            start=True, stop=True)
                    e = e_pool.tile([P, 1024], BF16, tag="e", name="e")
                    es[(kt, h)] = e
                    nc.scalar.activation(
                        e[:, cs:1024], sc[:, cs:1024],
                        mybir.ActivationFunctionType.Exp, scale=SCALE)
                    if kt >= 8 * G:
                        # diagonal 128-block: keep (q_rel - k_rel) >= 0
                        nc.gpsimd.affine_select(
                            out=e[:, cs:cs + P], in_=e[:, cs:cs + P],
                            compare_op=mybir.AluOpType.is_ge, fill=0.0,
                            base=0, pattern=[[1, P]], channel_multiplier=-1)

            def attnv(kt):
                q0 = max(0, kt - 8 * G)
                # sums partials for both heads share one rotating slot
                # ([0:8] = h0, [8:16] = h1) -> one DVE accumulate per kt
                sp_acc = op_slot()
                for h in range(2):
                    hh = 2 * p2 + h
                    e = es.pop((kt, h))
                    for qsub in range(q0, 8):
                        eblk = e[:, qsub * P:(qsub + 1) * P]
                        nc.tensor.matmul(
                            cx[h][:, qsub], eblk, v[:, kt, 64 * hh:64 * hh + 64],
                            start=(kt == 0 and qsub == 0),
                            stop=(kt == 8 * G + 7 and qsub == 7),
                            skip_group_check=True)
                    for qsub in range(q0, 8):
                        eblk = e[:, qsub * P:(qsub + 1) * P]
                        nc.tensor.matmul(
                            sp_acc[:, 2 * qsub + h:2 * qsub + h + 1], eblk,
                            ones_sb[:], start=(qsub == q0 and h == 0),
                            stop=(qsub == 7 and h == 1),
                            skip_group_check=True)
                if kt == 0:
                    nc.vector.tensor_copy(sums[:], sp_acc[:, 0:16])
                else:
                    nc.vector.tensor_tensor(
                        sums[:, 2 * q0:16], sums[:, 2 * q0:16],
                        sp_acc[:, 2 * q0:16], mybir.AluOpType.add)

            def finisher(j, pe_tp=False):
                qt = 8 * G + j
                rcp = rcp_pool.tile([P, 2], F32, tag="rcp", name="rcp")
                nc.vector.reciprocal_approx_fast(rcp[:], sums[:, 2 * j:2 * j + 2])
                ctx_n = ctxn_pool.tile([P, P], BF16, tag="ctxn", name="ctx_n")
                for h in range(2):
                    nc.vector.tensor_scalar(
                        ctx_n[:, 64 * h:64 * h + 64], cx[h][:, j],
                        rcp[:, h:h + 1], None, mybir.AluOpType.mult)
                if pe_tp:
                    # tail path: PE transpose + engine copy skips the
                    # ~1.5us DMA-XBAR setup+completion latency. ACT may
                    # only be used once no exp emission can follow
                    # (head-of-line blocking on the ACT queue).
                    tp = op_slot()[:, 0:64].bitcast(BF16)
                    nc.tensor.transpose(tp, ctx_n[:], ident_sb[:])
                    if pe_tp == "act":
                        nc.scalar.copy(ctxT[p2][qt][:], tp)
                    else:
                        nc.vector.tensor_copy(ctxT[p2][qt][:], tp)
                else:
                    nc.sync.dma_start_transpose(ctxT[p2][qt][:], ctx_n[:])

            for step in range(n_steps):
                if step < nkt:
                    scores_exp(step)
                for f in pinned.get(step, ()):
                    f()
                for f in share[step]:
                    f()
                if step > 0:
                    attnv(step - 1)
                    j = step - 1 - 8 * G
                    if 0 <= j < 8:
                        tp = False
                        if tail:
                            tp = "act" if j >= 6 else "dve"
                        finisher(j, pe_tp=tp)
                for f in post.get(step, ()):
                    f()

        # ---------- master schedule ----------
        def qk_half(p2, proj, qg, half):
            return lambda: emit_qk_half(p2, proj, qg, half)

        # A: prologue — just enough for sweep B to start (v nt0 first:
        # it only needs the tiny first x block + wv)
        emit_v(0)
        for qg in range(2):
            emit_qk(0, "q", qg)
        emit_qk_half(0, "k", 0, 0)

        # B: pair0 G0; pinned: v nt1-7 (attnv at step nt+1 needs v nt)
        # and k chunks at the exact steps their scores need them (B's own
        # kt2/4/6 + sweep C's kt0/2); q p1 spread over the rest
        injB = [qk_half(1, "q", 0, 0), qk_half(1, "q", 0, 1),
                qk_half(1, "q", 1, 0), qk_half(1, "q", 1, 1)]
        pinB = {nt: [lambda nt=nt: emit_v(nt)] for nt in range(1, 8)}
        pinB.setdefault(0, []).append(qk_half(0, "k", 0, 1))
        pinB.setdefault(1, []).append(qk_half(0, "k", 1, 0))
        pinB.setdefault(2, []).append(qk_half(0, "k", 1, 1))
        pinB.setdefault(3, []).append(qk_half(1, "k", 0, 0))
        pinB.setdefault(4, []).append(qk_half(1, "k", 0, 1))
        emit_sweep(0, 0, injB, pinB)

        # C: pair1 G0; pinned k p1 g1 (C kt4/6); inject p0 q/k G1 for
        # sweep D; post: out-proj qt0-3 (ctxT p0 from B, p1 this sweep)
        injC = [qk_half(0, pr, qg, h) for pr in ("q", "k")
                for qg in (2, 3) for h in (0, 1)]
        pinC = {0: [qk_half(1, "k", 1, 0)], 1: [qk_half(1, "k", 1, 1)]}
        postC = {4 + qt: [lambda qt=qt: emit_outproj(qt)] for qt in range(5)}
        emit_sweep(1, 0, injC, pinC, postC)

        # D: pair0 G1; pinned v nt8-15; inject p1 q/k G1 + out-proj qt4-7
        injD = [qk_half(1, pr, qg, h) for pr in ("q", "k")
                for qg in (2, 3) for h in (0, 1)]
        injD += [lambda qt=qt: emit_outproj(qt) for qt in range(4, 8)]
        pinD = {nt - 1: [lambda nt=nt: emit_v(nt)] for nt in range(8, 16)}
        emit_sweep(0, 1, injD, pinD)

        # E: pair1 G1; out-proj qt 8+j after this sweep's finisher j
        # (finisher j runs at step 9+j)
        postE = {}
        for j in range(8):
            postE.setdefault(min(10 + j, 16), []).append(
                lambda qt=8 + j, t=(j >= 5): emit_outproj(qt, tail=t))
        emit_sweep(1, 1, [], {}, postE, tail=True)

    nc.compile()
    return nc


_CACHE: dict = {}


def _get_nc() -> bass.Bass:
    """Module for simulation/inspection. Kept separate from the execution
    module: TimelineSim mutates state in a way that corrupts later runs."""
    if "nc_sim" not in _CACHE:
        _CACHE["nc_sim"] = build_nc()
    return _CACHE["nc_sim"]


def _get_nc_exec() -> bass.Bass:
    if "nc_exec" not in _CACHE:
        _CACHE["nc_exec"] = build_nc()
    return _CACHE["nc_exec"]


def make_in_maps(X, Wq, Wk, Wv, Wo):
    xts = [
        np.ascontiguousarray(
            np.asarray(X[b], dtype=np.float32).T).astype(ml_dtypes.bfloat16)
        for b in range(2)
    ]
    in_maps = []
    for c in range(NCORES):
        b, hg = c // 4, c % 4
        sl = slice(hg * DPC, (hg + 1) * DPC)
        in_maps.append({
            "x": xts[b],
            "wq": np.ascontiguousarray(Wq[:, sl]).astype(ml_dtypes.bfloat16),
            "wk": np.ascontiguousarray(Wk[:, sl]).astype(ml_dtypes.bfloat16),
            "wv": np.ascontiguousarray(Wv[:, sl]).astype(ml_dtypes.bfloat16),
            "wo": np.ascontiguousarray(Wo[sl, :]).astype(ml_dtypes.bfloat16),
        })
    return in_maps


def run_spmd(X, Wq, Wk, Wv, Wo, bo, **run_kwargs):
    nc = _get_nc_exec()
    in_maps = make_in_maps(X, Wq, Wk, Wv, Wo)
    res = run_bass_kernel_spmd(nc, in_maps, core_ids=list(range(NCORES)), **run_kwargs)
    acc = np.zeros((2, N, D), dtype=np.float32)
    for c, r in enumerate(res.results):
        acc[c // 4] += np.asarray(r["out"], dtype=np.float32)
    acc += np.asarray(bo, dtype=np.float32)
    return acc, res


def kernel(X, Wq, Wk, Wv, Wo, bo):
    out, _ = run_spmd(X, Wq, Wk, Wv, Wo, bo)
    return out


# Build the execution module eagerly at import: building AFTER a
# TimelineSim run in the same process produces a corrupted module
# (the simulator mutates global state that the builder reads).
_get_nc_exec()


# revision 39
# speedup vs baseline: 1.0178x; 1.0178x over previous
"""Trainium2 Bass kernel for causal multi-head attention.

Problem: X[2, 2048, 1024] fp32, 16 heads x head_dim 64, causal softmax,
out = softmax(QK^T/sqrt(64)) V @ Wo + bo.

Sharding: batch x head tensor parallel. Core c gets batch c//4 and the 4
heads 4*(c%4)..4*(c%4)+4 (two "pairs" of 2 heads): Wq/Wk/Wv column slices
[1024, 256], Wo row slice [256, 1024]. X^T (host pre-transposed, bf16)
replicated within each batch group. Each core returns a bf16 partial
[2048, 1024]; host sums 4 partials per batch + bias in fp32.

Per-core dataflow (all bf16 matmuls; cost model charges ap_size rows only):
  Q^T,K^T [128, 2048]  <- W-stationary MMs over X^T   (PSUM->SBUF bf16)
  V[k, 256]            <- X^T-stationary MMs -> natural [k, 4*64] layout
  S^T[k, q]            <- per (pair, 1024-q-group, kt): K^T chunk x Q^T
  e = exp(scale*S^T)   <- one wide ACT instr per (head, kt), diag masked
                          on Pool (affine_select)
  ctx[q, dh]           <- flipped MMs: lhsT = e block [128k, 128q]
                          (stationary), rhs = V slice; PSUM stripes
                          accumulate across kt
  sums                 <- ap-1 MMs (rhs = ones col) into rotating PSUM
                          quarter-slots, DVE-accumulated into SBUF
  normalize            <- DVE reciprocal of sums + per-partition
                          tensor_scalar mult -> ctx_n [128q, 128dh] bf16
  ctxT                 <- DMA-XBAR transpose ctx_n -> per-qt ctxT [dh, q]
  out[q, 1024]         <- ctxT (lhsT) x Wo chunks, 2-chunk PSUM accum,
                          bf16 partial DMA'd out (SP queue)

PSUM banks (8): scores h0 (2), scores h1 (2), ctx h0 (1), ctx h1 (1),
op0 (1), op1 (1). op0/op1 carry 4 rotating 256-col quarter-slots shared
by QKV projection chunks, out-projection quarters, and sums partials.

PSUM accumulation semantics (hardware-verified): start=True clears the
accumulate-bits of the WHOLE bank; each address's first write after that
overwrites, later writes accumulate. So every accumulation group in a
bank must be contiguous on the PE queue and start with start=True, OR
(ctx stripes) the bank gets exactly one start for the whole sweep.

Emission is software-pipelined: attn@V lags scores by one kt step, and
QKV / out-projection chunks are injected between attention steps so the
PE fills the gaps while ACT (the exp engine) streams.
"""

from contextlib import ExitStack

import numpy as np
import ml_dtypes

import concourse.bass as bass
import concourse.mybir as mybir
import concourse.tile as tile
from concourse import bacc
from concourse.bass_utils import run_bass_kernel_spmd
from concourse.masks import make_identity

N, D = 2048, 1024
NCORES = 8
HPC = 4                      # heads per core
DPC = HPC * 64               # projection cols per core = 256
DH = 64
P = 128
DC = D // P                  # 8 contraction chunks
NT = N // P                  # 16 n/k tiles
SCALE = 1.0 / float(np.sqrt(DH))

BF16 = mybir.dt.bfloat16
F32 = mybir.dt.float32


def build_nc() -> bass.Bass:
    nc = bacc.Bacc("TRN2", target_bir_lowering=False, debug=False)

    x = nc.dram_tensor("x", [D, N], BF16, kind="ExternalInput")
    wq = nc.dram_tensor("wq", [D, DPC], BF16, kind="ExternalInput")
    wk = nc.dram_tensor("wk", [D, DPC], BF16, kind="ExternalInput")
    wv = nc.dram_tensor("wv", [D, DPC], BF16, kind="ExternalInput")
    wo = nc.dram_tensor("wo", [DPC, D], BF16, kind="ExternalInput")
    out = nc.dram_tensor("out", [N, D], BF16, kind="ExternalOutput")

    with tile.TileContext(nc) as tc, ExitStack() as ctx:
        consts = ctx.enter_context(tc.tile_pool(name="consts", bufs=1))
        xt_pool = ctx.enter_context(tc.tile_pool(name="xt", bufs=1))
        qk_pool = ctx.enter_context(tc.tile_pool(name="qk", bufs=1))
        v_pool = ctx.enter_context(tc.tile_pool(name="v", bufs=1))
        e_pool = ctx.enter_context(tc.tile_pool(name="e", bufs=8))
        ctxT_pool = ctx.enter_context(tc.tile_pool(name="ctxT", bufs=1))
        ctxn_pool = ctx.enter_context(tc.tile_pool(name="ctxn", bufs=4))
        sums_pool = ctx.enter_context(tc.tile_pool(name="sums", bufs=2))
        rcp_pool = ctx.enter_context(tc.tile_pool(name="rcp", bufs=4))
        o_pool = ctx.enter_context(tc.tile_pool(name="o", bufs=6))
        ps = ctx.enter_context(tc.tile_pool(name="ps", bufs=1, space="PSUM"))

        # --- constants / weights + X^T, split across the two HWDGE queues
        # (SP + ACT) so the startup-critical pieces land early ---
        wq_sb = consts.tile([P, DC, DPC], BF16, tag="wq")
        wk_sb = consts.tile([P, DC, DPC], BF16, tag="wk")
        wv_sb = consts.tile([P, DC, DPC], BF16, tag="wv")
        wo_sb = consts.tile([P, 2, D], BF16, tag="wo")
        ones_sb = consts.tile([P, 1], BF16, tag="ones")
        ident_sb = consts.tile([P, P], BF16, tag="ident")
        nc.gpsimd.memset(ones_sb[:], 1.0)
        make_identity(nc, ident_sb[:])

        xt = xt_pool.tile([P, DC, N], BF16, tag="xt")
        xrr = x[:].rearrange("(dc p) n -> p dc n", p=P)
        # HWDGE setup (~625ns) does not overlap the transfer on a queue,
        # so use few, large DMAs. ACT queue: weights (wq first). SP queue:
        # X^T in increasing-n chunks; first mm needs only wq + x[0:256].
        nc.scalar.dma_start(wq_sb[:], wq[:].rearrange("(dc p) m -> p dc m", p=P))
        nc.scalar.dma_start(wv_sb[:], wv[:].rearrange("(dc p) m -> p dc m", p=P))
        nc.scalar.dma_start(wk_sb[:], wk[:].rearrange("(dc p) m -> p dc m", p=P))
        nc.scalar.dma_start(wo_sb[:], wo[:].rearrange("(c p) d -> p c d", p=P))
        for c0, c1 in ((0, 256), (256, 512), (512, 1024), (1024, 1536), (1536, 2048)):
            nc.sync.dma_start(xt[:, :, c0:c1], xrr[:, :, c0:c1])

        # SBUF destinations
        qT = [qk_pool.tile([P, N], BF16, tag=f"qT{p}", name=f"qT{p}") for p in range(2)]
        kT = [qk_pool.tile([P, N], BF16, tag=f"kT{p}", name=f"kT{p}") for p in range(2)]
        v = v_pool.tile([P, NT, DPC], BF16, tag="v")
        # per-qt ctxT tiles: the DMA-XBAR transpose writes whole tiles so
        # write coverage is registered exactly.
        ctxT = [[ctxT_pool.tile([P, P], BF16, tag=f"ctxT{p}_{qt}",
                                name=f"ctxT{p}_{qt}") for qt in range(NT)]
                for p in range(2)]

        # ---------- rotating PSUM quarter-slots ----------
        op_banks = [ps.tile([P, 2, 256], F32, tag=f"op{i}", name=f"op{i}")
                    for i in range(2)]
        rr = [0]

        def op_slot():
            i = rr[0]
            rr[0] = (i + 1) % 4
            return op_banks[i // 2][:, i % 2]

        def emit_qk_half(p2, proj, qg, half):
            """Project q or k for pair p2, n-columns [qg*512+half*256 ...+256]."""
            w_sb = wq_sb if proj == "q" else wk_sb
            dst = qT[p2] if proj == "q" else kT[p2]
            sl = slice(qg * 512 + half * 256, qg * 512 + half * 256 + 256)
            acc = op_slot()
            for dc in range(DC):
                nc.tensor.matmul(
                    acc, w_sb[:, dc, 128 * p2:128 * p2 + 128],
                    xt[:, dc, sl], start=(dc == 0), stop=(dc == DC - 1),
                    skip_group_check=True)
            nc.vector.tensor_copy(dst[:, sl], acc)

        def emit_qk(p2, proj, qg):
            for half in range(2):
                emit_qk_half(p2, proj, qg, half)

        def emit_v(nt):
            """V natural layout for n-tile nt: [128 k, 256 dh] (all 4 heads)."""
            acc = op_slot()
            for dc in range(DC):
                nc.tensor.matmul(
                    acc, xt[:, dc, nt * P:(nt + 1) * P],
                    wv_sb[:, dc, :], start=(dc == 0), stop=(dc == DC - 1),
                    skip_group_check=True)
            nc.vector.tensor_copy(v[:, nt, :], acc)

        def emit_outproj(qt, tail=False):
            """Out-projection for q-tile qt: 4 quarter-columns of 256.
            tail=True: past the last exp — alternate copies onto the
            otherwise-idle ACT engine."""
            for half in range(2):
                o_sb = o_pool.tile([P, 512], BF16, tag="o", name="o_sb")
                for sub in range(2):
                    quarter = 2 * half + sub
                    acc = op_slot()
                    for p2 in range(2):
                        nc.tensor.matmul(
                            acc, ctxT[p2][qt][:],
                            wo_sb[:, p2, quarter * 256:(quarter + 1) * 256],
                            start=(p2 == 0), stop=(p2 == 1),
                            skip_group_check=True)
                    dst = o_sb[:, sub * 256:(sub + 1) * 256]
                    if tail and sub == 1:
                        nc.scalar.copy(dst, acc)
                    else:
                        nc.vector.tensor_copy(dst, acc)
                nc.sync.dma_start(
                    out[qt * P:(qt + 1) * P, half * 512:(half + 1) * 512],
                    o_sb[:])

        # ---------- attention sweep ----------
        def emit_sweep(p2, G, inject, pinned, post=None, tail=False):
            """Attention for pair p2 over q in [G*1024, (G+1)*1024).
            inject: closures spread over steps; pinned: step -> closures
            run before attnv; post: step -> closures run after that
            step's finisher (consumers of this sweep's ctxT go here).
            tail=True: last sweep of the kernel — use the low-latency
            PE-transpose finisher for the final q-tiles."""
            post = post or {}
            nkt = 8 * G + 8
            cx = [ps.tile([P, 8, DH], F32, tag=f"cx{h}", name=f"cx{h}")
                  for h in range(2)]
            # SBUF softmax-sum accumulators, qsub-major h-minor [128, 16]
            sums = sums_pool.tile([P, 16], F32, tag="sums", name="sums")
            es = {}
            n_steps = nkt + 1
            share = [[] for _ in range(n_steps)]
            for i, f in enumerate(inject):
                share[min(1 + i * n_steps // max(1, len(inject)), n_steps - 1)].append(f)

            def scores_exp(kt):
                cs = max(0, kt * P - G * 1024)
                for h in range(2):
                    sc = ps.tile([P, 1024], F32, tag=f"sc{h}", name=f"sc{h}")
                    for c0, c1 in ((cs, 512), (max(cs, 512), 1024)):
                        if c0 >= c1:
                            continue
                        nc.tensor.matmul(
                            sc[:, c0:c1],
                            kT[p2][64 * h:64 * h + 64, kt * P:(kt + 1) * P],
                            qT[p2][64 * h:64 * h + 64, G * 1024 + c0:G * 1024 + c1],
                            start=True, stop=True)
                    e = e_pool.tile([P, 1024], BF16, tag="e", name="e")
                    es[(kt, h)] = e
                    nc.scalar.activation(
                        e[:, cs:1024], sc[:, cs:1024],
                        mybir.ActivationFunctionType.Exp, scale=SCALE)
                    if kt >= 8 * G:
                        # diagonal 128-block: keep (q_rel - k_rel) >= 0
                        nc.gpsimd.affine_select(
                            out=e[:, cs:cs + P], in_=e[:, cs:cs + P],
                            compare_op=mybir.AluOpType.is_ge, fill=0.0,
                            base=0, pattern=[[1, P]], channel_multiplier=-1)

            def attnv(kt):
                q0 = max(0, kt - 8 * G)
                # sums partials for both heads share one rotating slot
                # ([0:8] = h0, [8:16] = h1) -> one DVE accumulate per kt
                sp_acc = op_slot()
                for h in range(2):
                    hh = 2 * p2 + h
                    e = es.pop((kt, h))
                    for qsub in range(q0, 8):
                        eblk = e[:, qsub * P:(qsub + 1) * P]
                        nc.tensor.matmul(
                            cx[h][:, qsub], eblk, v[:, kt, 64 * hh:64 * hh + 64],
                            start=(kt == 0 and qsub == 0),
                            stop=(kt == 8 * G + 7 and qsub == 7),
                            skip_group_check=True)
                    for qsub in range(q0, 8):
                        eblk = e[:, qsub * P:(qsub + 1) * P]
                        nc.tensor.matmul(
                            sp_acc[:, 2 * qsub + h:2 * qsub + h + 1], eblk,
                            ones_sb[:], start=(qsub == q0 and h == 0),
                            stop=(qsub == 7 and h == 1),
                            skip_group_check=True)
                if kt == 0:
                    nc.vector.tensor_copy(sums[:], sp_acc[:, 0:16])
                else:
                    nc.vector.tensor_tensor(
                        sums[:, 2 * q0:16], sums[:, 2 * q0:16],
                        sp_acc[:, 2 * q0:16], mybir.AluOpType.add)

            def finisher(j, pe_tp=False):
                qt = 8 * G + j
                rcp = rcp_pool.tile([P, 2], F32, tag="rcp", name="rcp")
                nc.vector.reciprocal_approx_fast(rcp[:], sums[:, 2 * j:2 * j + 2])
                ctx_n = ctxn_pool.tile([P, P], BF16, tag="ctxn", name="ctx_n")
                for h in range(2):
                    nc.vector.tensor_scalar(
                        ctx_n[:, 64 * h:64 * h + 64], cx[h][:, j],
                        rcp[:, h:h + 1], None, mybir.AluOpType.mult)
                if pe_tp:
                    # tail path: PE transpose + engine copy skips the
                    # ~1.5us DMA-XBAR setup+completion latency. ACT may
                    # only be used once no exp emission can follow
                    # (head-of-line blocking on the ACT queue).
                    tp = op_slot()[:, 0:64].bitcast(BF16)
                    nc.tensor.transpose(tp, ctx_n[:], ident_sb[:])
                    if pe_tp == "act":
                        nc.scalar.copy(ctxT[p2][qt][:], tp)
                    else:
                        nc.vector.tensor_copy(ctxT[p2][qt][:], tp)
                else:
                    nc.sync.dma_start_transpose(ctxT[p2][qt][:], ctx_n[:])

            for step in range(n_steps):
                if step < nkt:
                    scores_exp(step)
                for f in pinned.get(step, ()):
                    f()
                for f in share[step]:
                    f()
                if step > 0:
                    attnv(step - 1)
                    j = step - 1 - 8 * G
                    if 0 <= j < 8:
                        tp = False
                        if tail:
                            tp = "act" if j >= 6 else "dve"
                        finisher(j, pe_tp=tp)
                for f in post.get(step, ()):
                    f()

        # ---------- master schedule ----------
        def qk_half(p2, proj, qg, half):
            return lambda: emit_qk_half(p2, proj, qg, half)

        # A: prologue — just enough for sweep B to start (v nt0 first:
        # it only needs the tiny first x block + wv)
        emit_v(0)
        for qg in range(2):
            emit_qk(0, "q", qg)
        emit_qk_half(0, "k", 0, 0)

        # B: pair0 G0; pinned: v nt1-7 (attnv at step nt+1 needs v nt)
        # and k chunks at the exact steps their scores need them (B's own
        # kt2/4/6 + sweep C's kt0/2); q p1 spread over the rest
        injB = [qk_half(1, "q", 0, 0), qk_half(1, "q", 0, 1),
                qk_half(1, "q", 1, 0), qk_half(1, "q", 1, 1)]
        pinB = {nt: [lambda nt=nt: emit_v(nt)] for nt in range(1, 8)}
        pinB.setdefault(0, []).append(qk_half(0, "k", 0, 1))
        pinB.setdefault(1, []).append(qk_half(0, "k", 1, 0))
        pinB.setdefault(2, []).append(qk_half(0, "k", 1, 1))
        pinB.setdefault(3, []).append(qk_half(1, "k", 0, 0))
        pinB.setdefault(4, []).append(qk_half(1, "k", 0, 1))
        emit_sweep(0, 0, injB, pinB)

        # C: pair1 G0; pinned k p1 g1 (C kt4/6); inject p0 q/k G1 for
        # sweep D; post: out-proj qt0-3 (ctxT p0 from B, p1 this sweep)
        injC = [qk_half(0, pr, qg, h) for pr in ("q", "k")
                for qg in (2, 3) for h in (0, 1)]
        pinC = {0: [qk_half(1, "k", 1, 0)], 1: [qk_half(1, "k", 1, 1)]}
        postC = {5 + qt: [lambda qt=qt: emit_outproj(qt)] for qt in range(4)}
        emit_sweep(1, 0, injC, pinC, postC)

        # D: pair0 G1; pinned v nt8-15; inject p1 q/k G1 + out-proj qt4-7
        injD = [qk_half(1, pr, qg, h) for pr in ("q", "k")
                for qg in (2, 3) for h in (0, 1)]
        injD += [lambda qt=qt: emit_outproj(qt) for qt in range(4, 8)]
        pinD = {nt - 1: [lambda nt=nt: emit_v(nt)] for nt in range(8, 16)}
        emit_sweep(0, 1, injD, pinD)

        # E: pair1 G1; out-proj qt 8+j after this sweep's finisher j
        # (finisher j runs at step 9+j)
        postE = {}
        for j in range(8):
            postE.setdefault(min(10 + j, 16), []).append(
                lambda qt=8 + j, t=(j >= 5): emit_outproj(qt, tail=t))
        emit_sweep(1, 1, [], {}, postE, tail=True)

    nc.compile()
    return nc


_CACHE: dict = {}


def _get_nc() -> bass.Bass:
    """Module for simulation/inspection. Kept separate from the execution
    module: TimelineSim mutates state in a way that corrupts later runs."""
    if "nc_sim" not in _CACHE:
        _CACHE["nc_sim"] = build_nc()
    return _CACHE["nc_sim"]


def _get_nc_exec() -> bass.Bass:
    if "nc_exec" not in _CACHE:
        _CACHE["nc_exec"] = build_nc()
    return _CACHE["nc_exec"]


def make_in_maps(X, Wq, Wk, Wv, Wo):
    xts = [
        np.ascontiguousarray(
            np.asarray(X[b], dtype=np.float32).T).astype(ml_dtypes.bfloat16)
        for b in range(2)
    ]
    in_maps = []
    for c in range(NCORES):
        b, hg = c // 4, c % 4
        sl = slice(hg * DPC, (hg + 1) * DPC)
        in_maps.append({
            "x": xts[b],
            "wq": np.ascontiguousarray(Wq[:, sl]).astype(ml_dtypes.bfloat16),
            "wk": np.ascontiguousarray(Wk[:, sl]).astype(ml_dtypes.bfloat16),
            "wv": np.ascontiguousarray(Wv[:, sl]).astype(ml_dtypes.bfloat16),
            "wo": np.ascontiguousarray(Wo[sl, :]).astype(ml_dtypes.bfloat16),
        })
    return in_maps


def run_spmd(X, Wq, Wk, Wv, Wo, bo, **run_kwargs):
    nc = _get_nc_exec()
    in_maps = make_in_maps(X, Wq, Wk, Wv, Wo)
    res = run_bass_kernel_spmd(nc, in_maps, core_ids=list(range(NCORES)), **run_kwargs)
    acc = np.zeros((2, N, D), dtype=np.float32)
    for c, r in enumerate(res.results):
        acc[c // 4] += np.asarray(r["out"], dtype=np.float32)
    acc += np.asarray(bo, dtype=np.float32)
    return acc, res


def kernel(X, Wq, Wk, Wv, Wo, bo):
    out, _ = run_spmd(X, Wq, Wk, Wv, Wo, bo)
    return out


# Build the execution module eagerly at import: building AFTER a
# TimelineSim run in the same process produces a corrupted module
# (the simulator mutates global state that the builder reads).
_get_nc_exec()
